# revision 5
# baseline (speedup 1.0000x reference)
"""Trainium2 Bass kernel for nn_ConceptLayer (B=8, S=4096, D=64).

out[b,i,k] = LN( x[b,i,:] + sum_{a,c} x[b,i,a] * s_pre[b,i,c] * W[k,a,c] )
s_pre[b,i,c] = sum_{j<i} x[b,j,c] / (i-j)^2

Sharding: data-parallel over batch — one batch element per NeuronCore (8 cores).

Per-core algorithm (all positions i tiled by 128, blocks of 512):
  Phase T: xT = x.T via PE transposes (for the replication matmuls).
  Phase A: s_pre2[c(+64dup), i] = sum_J x2[J].T @ TTS-slice  (Toeplitz strip,
           32 distinct 128x128 blocks + 3 zero blocks; causal masked diag).
  Phase B: for each 512-block of i and each 128-chunk g of (a,c)=a*64+c:
           rep_g = E_g.T @ xT  (PE replicates xT rows 2g,2g+1 64x each)
           outerT_g = rep_g * s_pre2   (DVE product, bf16 out)
           psum_out[i-tile] += outerT_g-slice.T @ W2T_g  (accumulate over g)
  Epilogue: r = out + x ; LayerNorm over k (bn_stats/bn_aggr, sqrt, recip);
           y = (r - mu) * rstd * gamma + beta ; DMA out.
"""

import sys

sys.path.insert(0, "/opt/trn_rl_repo")

import numpy as np
import ml_dtypes

import concourse.bass as bass
import concourse.mybir as mybir
from concourse.tile import TileContext
from concourse.vector_clock import ScopedClock
from concourse.bass_utils import run_bass_kernel_spmd
from concourse.masks import make_identity

B, S, D = 8, 4096, 64
LN_EPS = 1e-3
P = 128
NT = S // P            # 32 i-tiles
NB = S // 512          # 8 i-blocks
NG = (D * D) // P      # 32 (a,c) chunks
NSTRIP = NB * 4 + 3    # 35 offset blocks in the Toeplitz strip

F32 = mybir.dt.float32
BF16 = mybir.dt.bfloat16
BF16_NP = ml_dtypes.bfloat16


# ---------------------------------------------------------------------------
# Workaround for walrus "Too many sync wait commands": this walrus build only
# accepts a single embedded sem wait per instruction. After Tile scheduling,
# split any instruction with N>1 waits into N-1 single-wait NOPs (same engine,
# placed just before it — identical blocking semantics) + the instruction
# keeping one wait.
def _split_multiwait(nc: bass.Bass, keep: int = 1):
    n = 0
    for fn in nc.m.functions:
        for bb in fn.blocks:
            insts = list(bb.instructions)
            out = []
            changed = False
            for inst in insts:
                si = inst.sync_info
                if si is not None and len(si.on_wait) > keep:
                    waits = list(si.on_wait)
                    for w in waits[:-keep]:
                        nop = mybir.InstNoOp(
                            name=f"WSPLIT-{n}", engine=inst.engine, ins=[], outs=[]
                        )
                        n += 1
                        nop.sync_info = mybir.SyncInfo(on_wait=[w], on_update=[])
                        out.append(nop)
                    inst.sync_info = mybir.SyncInfo(
                        on_wait=waits[-keep:], on_update=list(si.on_update)
                    )
                    changed = True
                out.append(inst)
            if changed:
                bb.instructions = out
    return n
# ---------------------------------------------------------------------------


def _host_constants(concept_map: np.ndarray):
    """Precompute host-side constant tensors (replicated across cores)."""
    # Toeplitz strip: TTS[q, 128*s + n] = f(128*(s-3) + n - q), f(v)=1/v^2 (v>0)
    q = np.arange(P)
    col = np.arange(NSTRIP * P)
    sblk, n_ = col // P, col % P
    v = 128 * (sblk[None, :] - 3) + n_[None, :] - q[:, None]
    tts = np.where(v > 0, 1.0 / np.maximum(v, 1).astype(np.float64) ** 2, 0.0)
    tts = tts.astype(np.float32)

    # W2T[a*64+c, k] = W[k, a, c]
    w2t = np.ascontiguousarray(
        concept_map.transpose(1, 2, 0).reshape(D * D, D)
    ).astype(np.float32)

    # E_all[c, 128*g + p] = 1 if c == 2g + p//64
    eall = np.zeros((D, NG * P), np.float32)
    g = np.arange(NG * P) // P
    p = np.arange(NG * P) % P
    eall[2 * g + p // 64, np.arange(NG * P)] = 1.0
    return (
        tts.astype(BF16_NP),
        w2t.astype(BF16_NP),
        eall.astype(BF16_NP),
    )


def _build_nc() -> bass.Bass:
    nc = bass.Bass("TRN2", target_bir_lowering=False, debug=False, num_devices=B)

    xb = nc.dram_tensor("xb", [S, D], F32, kind="ExternalInput")
    x2b = nc.dram_tensor("x2b", [S, 2 * D], BF16, kind="ExternalInput")
    tts_d = nc.dram_tensor("tts", [P, NSTRIP * P], BF16, kind="ExternalInput")
    w2t_d = nc.dram_tensor("w2t", [D * D, D], BF16, kind="ExternalInput")
    eall_d = nc.dram_tensor("eall", [D, NG * P], BF16, kind="ExternalInput")
    gamma_d = nc.dram_tensor("gamma", [D], F32, kind="ExternalInput")
    beta_d = nc.dram_tensor("beta", [D], F32, kind="ExternalInput")
    y_d = nc.dram_tensor("y", [S, D], F32, kind="ExternalOutput")

    with TileContext(nc) as tc:
        with (
            tc.tile_pool(name="singles", bufs=1) as singles,
            tc.tile_pool(name="eplg", bufs=4) as eplg,
        ):
            # ---- resident SBUF tiles -------------------------------------
            xf = singles.tile([P, NT, D], F32)         # x, 128Jxc tiles
            nc.sync.dma_start(
                out=xf, in_=xb.rearrange("(j p) c -> p j c", p=P)
            )
            x2t = singles.tile([P, NT, 2 * D], BF16)   # [x|x] bf16 tiles
            nc.sync.dma_start(
                out=x2t, in_=x2b.rearrange("(j p) c -> p j c", p=P)
            )
            tts = singles.tile([P, NSTRIP * P], BF16)
            nc.sync.dma_start(out=tts, in_=tts_d[:])
            w2t = singles.tile([P, NG, D], BF16)
            nc.sync.dma_start(
                out=w2t, in_=w2t_d.rearrange("(g p) k -> p g k", p=P)
            )
            eall = singles.tile([D, NG, P], BF16)
            nc.sync.dma_start(
                out=eall, in_=eall_d.rearrange("c (g p) -> c g p", p=P)
            )
            gam = singles.tile([P, D], F32)
            nc.sync.dma_start(
                out=gam,
                in_=bass.AP(
                    tensor=gamma_d.ap().tensor,
                    offset=gamma_d.ap().offset,
                    ap=[[0, P], [1, D]],
                ),
            )
            bet = singles.tile([P, D], F32)
            nc.sync.dma_start(
                out=bet,
                in_=bass.AP(
                    tensor=beta_d.ap().tensor,
                    offset=beta_d.ap().offset,
                    ap=[[0, P], [1, D]],
                ),
            )
            eps_t = singles.tile([P, 1], F32)
            nc.vector.memset(eps_t, LN_EPS)
            ident = singles.tile([P, P], F32)
            make_identity(nc, ident)

            xT = singles.tile([D, NT, P], BF16)        # x.T, bf16
            s2dup = singles.tile([P, NB, 1024], F32)   # s_pre stacked+dup'd
            outerT = singles.tile([P, NG * 512], BF16, tag="outerT")

            # ---- Phase T: transpose x (PE) -------------------------------
            with tc.tile_pool(name="pt", bufs=2, space="PSUM") as pt:
                for J in range(NT):
                    ps = pt.tile([D, P], F32)
                    nc.tensor.transpose(ps, in_=xf[:, J, :], identity=ident)
                    nc.scalar.copy(out=xT[:, J, :], in_=ps)

            # ---- Phase A: s_pre (PE) -------------------------------------
            with tc.tile_pool(name="pa", bufs=2, space="PSUM") as pa:
                for ib in range(NB):
                    ps = pa.tile([P, 512], F32)
                    for J in range(4 * ib + 4):
                        s0 = 4 * ib - J + 3
                        nc.tensor.matmul(
                            ps,
                            lhsT=x2t[:, J, :],
                            rhs=tts[:, 128 * s0 : 128 * s0 + 512],
                            start=(J == 0),
                            stop=(J == 4 * ib + 3),
                        )
                    nc.vector.tensor_copy(out=s2dup[:, ib, 0:512], in_=ps)
                    nc.vector.tensor_copy(out=s2dup[:, ib, 512:1024], in_=ps)

            # ---- Phase B: rep, product, bilinear, LN ---------------------
            with (
                tc.tile_pool(name="prep", bufs=3, space="PSUM") as prep,
                tc.tile_pool(name="pout", bufs=2, space="PSUM") as pout,
            ):
                for ib in range(NB):
                    isl = slice(512 * ib, 512 * (ib + 1))
                    for gp in range(NG // 2):
                        psr = prep.tile([P, 1024], F32)
                        for h in (0, 1):
                            g = 2 * gp + h
                            nc.tensor.matmul(
                                psr[:, 512 * h : 512 * (h + 1)],
                                lhsT=eall[:, g, :],
                                rhs=xT[:, 4 * ib : 4 * ib + 4, :].rearrange(
                                    "c t p -> c (t p)"
                                ),
                                start=True,
                                stop=True,
                            )
                        nc.vector.tensor_mul(
                            outerT[:, 1024 * gp : 1024 * (gp + 1)],
                            psr,
                            s2dup[:, ib, :],
                        )
                    for t in range(4):
                        pso = pout.tile([P, D], F32)
                        for g in range(NG):
                            nc.tensor.matmul(
                                pso,
                                lhsT=outerT[:, 512 * g + 128 * t : 512 * g + 128 * (t + 1)],
                                rhs=w2t[:, g, :],
                                start=(g == 0),
                                stop=(g == NG - 1),
                            )
                        # epilogue: residual + LayerNorm
                        it = 4 * ib + t
                        r = eplg.tile([P, D], F32, tag="r")
                        nc.vector.tensor_add(r, pso, xf[:, it, :])
                        stats = eplg.tile([P, 6], F32, tag="stats")
                        nc.vector.bn_stats(out=stats, in_=r)
                        mv = eplg.tile([P, 2], F32, tag="mv")
                        nc.vector.bn_aggr(out=mv, in_=stats)
                        rstd = eplg.tile([P, 1], F32, tag="rstd")
                        nc.scalar.activation(
                            out=rstd,
                            in_=mv[:, 1:2],
                            func=mybir.ActivationFunctionType.Sqrt,
                            bias=eps_t,
                            scale=1.0,
                        )
                        nc.vector.reciprocal(out=rstd, in_=rstd)
                        y = eplg.tile([P, D], F32, tag="y")
                        nc.vector.tensor_scalar(
                            out=y,
                            in0=r,
                            scalar1=mv[:, 0:1],
                            scalar2=rstd,
                            op0=mybir.AluOpType.subtract,
                            op1=mybir.AluOpType.mult,
                        )
                        nc.vector.tensor_mul(y, y, gam)
                        nc.vector.tensor_add(y, y, bet)
                        nc.sync.dma_start(
                            out=y_d[128 * it : 128 * (it + 1), :], in_=y
                        )
    _split_multiwait(nc)
    return nc


_CACHED = {}


def kernel(**inputs: np.ndarray) -> np.ndarray:
    x = np.asarray(inputs["x"], np.float32)
    w = np.asarray(inputs["concept_map"], np.float32)
    gamma = np.asarray(inputs["gamma"], np.float32)
    beta = np.asarray(inputs["beta"], np.float32)
    assert x.shape == (B, S, D)

    tts, w2t, eall = _host_constants(w)
    if "nc" not in _CACHED:
        _CACHED["nc"] = _build_nc()
    nc = _CACHED["nc"]

    in_maps = []
    for b in range(B):
        xb = np.ascontiguousarray(x[b])
        in_maps.append(
            {
                "xb": xb,
                "x2b": np.concatenate([xb, xb], axis=1).astype(BF16_NP),
                "tts": tts,
                "w2t": w2t,
                "eall": eall,
                "gamma": gamma,
                "beta": beta,
            }
        )
    res = run_bass_kernel_spmd(nc, in_maps, core_ids=list(range(B)))
    return np.stack([res.results[b]["y"] for b in range(B)], axis=0)


if __name__ == "__main__":
    rng = np.random.default_rng(0)
    ins = {
        "x": rng.standard_normal((B, S, D), dtype=np.float32),
        "concept_map": (rng.standard_normal((D, D, D)) * 0.02).astype(np.float32),
        "gamma": np.ones(D, np.float32),
        "beta": np.zeros(D, np.float32),
    }
    y = kernel(**ins)
    print("ran", y.shape, y.dtype)


# revision 6
# speedup vs baseline: 5642.3318x; 5642.3318x over previous
"""Trainium2 Bass kernel for nn_ConceptLayer (B=8, S=4096, D=64).

out[b,i,k] = LN( x[b,i,:] + sum_{a,c} x[b,i,a] * s_pre[b,i,c] * W[k,a,c] )
s_pre[b,i,c] = sum_{j<i} x[b,j,c] / (i-j)^2

Sharding: data-parallel over batch — one batch element per NeuronCore (8 cores).

Per-core algorithm (positions i tiled by 128, blocks of 512):
  Phase T: xT = x.T via PE transposes (for the replication matmuls).
  Phase A: s_pre2[c(+64dup), i] = sum_J x2[J].T @ TTS-slice  (Toeplitz strip,
           32 distinct 128x128 blocks + 3 zero blocks; causal masked diag).
  Phase B: for each 512-block of i and each 128-chunk g of (a,c)=a*64+c:
           rep_g = E_g.T @ xT  (PE replicates xT rows 2g,2g+1 64x each)
           outerT_g = rep_g * s_pre2   (DVE product, bf16 out)
           psum_out[i-tile] += outerT_g-slice.T @ W2T_g  (accumulate over g)
  Epilogue: r = out + x ; LayerNorm over k (bn_stats/bn_aggr, sqrt, recip);
           y = (r - mu) * rstd * gamma + beta ; DMA out.
"""

import sys

sys.path.insert(0, "/opt/trn_rl_repo")

import numpy as np
import ml_dtypes

import concourse.bass as bass
import concourse.mybir as mybir
from concourse.tile import TileContext
from concourse.bass_utils import run_bass_kernel_spmd
from concourse.masks import make_identity

B, S, D = 8, 4096, 64
LN_EPS = 1e-3
P = 128
NT = S // P            # 32 i-tiles
NB = S // 512          # 8 i-blocks
NG = (D * D) // P      # 32 (a,c) chunks
NSTRIP = NB * 4 + 3    # 35 offset blocks in the Toeplitz strip

F32 = mybir.dt.float32
BF16 = mybir.dt.bfloat16
BF16_NP = ml_dtypes.bfloat16


# ---------------------------------------------------------------------------
# Workaround for walrus "Too many sync wait commands": this walrus build only
# accepts a single embedded sem wait per instruction. After Tile scheduling,
# split any instruction with N>1 waits into N-1 single-wait NOPs (same engine,
# placed just before it — identical blocking semantics) + the instruction
# keeping one wait.
def _split_multiwait(nc: bass.Bass, keep: int = 1):
    n = 0
    for fn in nc.m.functions:
        for bb in fn.blocks:
            insts = list(bb.instructions)
            out = []
            changed = False
            for inst in insts:
                si = inst.sync_info
                if si is not None and len(si.on_wait) > keep:
                    waits = list(si.on_wait)
                    for w in waits[:-keep]:
                        nop = mybir.InstNoOp(
                            name=f"WSPLIT-{n}", engine=inst.engine, ins=[], outs=[]
                        )
                        n += 1
                        nop.sync_info = mybir.SyncInfo(on_wait=[w], on_update=[])
                        out.append(nop)
                    inst.sync_info = mybir.SyncInfo(
                        on_wait=waits[-keep:], on_update=list(si.on_update)
                    )
                    changed = True
                out.append(inst)
            if changed:
                bb.instructions = out
    return n
# ---------------------------------------------------------------------------


def _host_constants(concept_map: np.ndarray):
    """Precompute host-side constant tensors (replicated across cores)."""
    # Toeplitz strip: TTS[q, 128*s + n] = f(128*(s-3) + n - q), f(v)=1/v^2 (v>0)
    q = np.arange(P)
    col = np.arange(NSTRIP * P)
    sblk, n_ = col // P, col % P
    v = 128 * (sblk[None, :] - 3) + n_[None, :] - q[:, None]
    tts = np.where(v > 0, 1.0 / np.maximum(v, 1).astype(np.float64) ** 2, 0.0)
    tts = tts.astype(np.float32)

    # W2T[a*64+c, k] = W[k, a, c]
    w2t = np.ascontiguousarray(
        concept_map.transpose(1, 2, 0).reshape(D * D, D)
    ).astype(np.float32)

    # E_all[c, 128*g + p] = 1 if c == 2g + p//64
    eall = np.zeros((D, NG * P), np.float32)
    g = np.arange(NG * P) // P
    p = np.arange(NG * P) % P
    eall[2 * g + p // 64, np.arange(NG * P)] = 1.0
    return (
        tts.astype(BF16_NP),
        w2t.astype(BF16_NP),
        eall.astype(BF16_NP),
    )


def _build_nc(reps: int = 1) -> bass.Bass:
    nc = bass.Bass("TRN2", target_bir_lowering=False, debug=False, num_devices=B)

    xb = nc.dram_tensor("xb", [S, D], F32, kind="ExternalInput")
    x2b = nc.dram_tensor("x2b", [S, 2 * D], BF16, kind="ExternalInput")
    tts_d = nc.dram_tensor("tts", [P, NSTRIP * P], BF16, kind="ExternalInput")
    w2t_d = nc.dram_tensor("w2t", [D * D, D], BF16, kind="ExternalInput")
    eall_d = nc.dram_tensor("eall", [D, NG * P], BF16, kind="ExternalInput")
    gamma_d = nc.dram_tensor("gamma", [D], F32, kind="ExternalInput")
    beta_d = nc.dram_tensor("beta", [D], F32, kind="ExternalInput")
    y_d = nc.dram_tensor("y", [S, D], F32, kind="ExternalOutput")

    with TileContext(nc) as tc:
        with (
            tc.tile_pool(name="singles", bufs=1) as singles,
            tc.tile_pool(name="eplg", bufs=4) as eplg,
            tc.tile_pool(name="ps_small", bufs=2, space="PSUM") as ps_small,
            tc.tile_pool(name="prep", bufs=2, space="PSUM") as prep,
            tc.tile_pool(name="pout", bufs=2, space="PSUM") as pout,
        ):

            def body():
                # ---- resident SBUF tiles ---------------------------------
                xf = singles.tile([P, NT, D], F32, tag="xf")
                nc.sync.dma_start(
                    out=xf, in_=xb.rearrange("(j p) c -> p j c", p=P)
                )
                x2t = singles.tile([P, NT, 2 * D], BF16, tag="x2t")
                nc.sync.dma_start(
                    out=x2t, in_=x2b.rearrange("(j p) c -> p j c", p=P)
                )
                tts = singles.tile([P, NSTRIP * P], BF16, tag="tts")
                nc.sync.dma_start(out=tts, in_=tts_d[:])
                w2t = singles.tile([P, NG, D], BF16, tag="w2t")
                nc.sync.dma_start(
                    out=w2t, in_=w2t_d.rearrange("(g p) k -> p g k", p=P)
                )
                eall = singles.tile([D, NG, P], BF16, tag="eall")
                nc.sync.dma_start(
                    out=eall, in_=eall_d.rearrange("c (g p) -> c g p", p=P)
                )
                gam = singles.tile([P, D], F32, tag="gam")
                nc.sync.dma_start(
                    out=gam,
                    in_=bass.AP(
                        tensor=gamma_d.ap().tensor,
                        offset=gamma_d.ap().offset,
                        ap=[[0, P], [1, D]],
                    ),
                )
                bet = singles.tile([P, D], F32, tag="bet")
                nc.sync.dma_start(
                    out=bet,
                    in_=bass.AP(
                        tensor=beta_d.ap().tensor,
                        offset=beta_d.ap().offset,
                        ap=[[0, P], [1, D]],
                    ),
                )
                eps_t = singles.tile([P, 1], F32, tag="eps")
                nc.vector.memset(eps_t, LN_EPS)
                ident = singles.tile([P, P], F32, tag="ident")
                make_identity(nc, ident)

                xT = singles.tile([D, NT, P], BF16, tag="xT")
                s2dup = singles.tile([P, NB, 1024], F32, tag="s2dup")
                outerT = singles.tile([P, NG * 512], BF16, tag="outerT")

                # ---- Phase T: transpose x (PE) ---------------------------
                for J in range(NT):
                    ps = ps_small.tile([D, P], F32, tag="ps_small")
                    nc.tensor.transpose(ps, in_=xf[:, J, :], identity=ident)
                    nc.scalar.copy(out=xT[:, J, :], in_=ps)

                # ---- Phase A: s_pre (PE) ---------------------------------
                for ib in range(NB):
                    ps = ps_small.tile([P, 512], F32, tag="ps_small")
                    for J in range(4 * ib + 4):
                        s0 = 4 * ib - J + 3
                        nc.tensor.matmul(
                            ps,
                            lhsT=x2t[:, J, :],
                            rhs=tts[:, 128 * s0 : 128 * s0 + 512],
                            start=(J == 0),
                            stop=(J == 4 * ib + 3),
                        )
                    nc.vector.tensor_copy(out=s2dup[:, ib, 0:512], in_=ps)
                    nc.vector.tensor_copy(out=s2dup[:, ib, 512:1024], in_=ps)

                # ---- Phase B: rep, product, bilinear, LN -----------------
                for ib in range(NB):
                    for gp in range(NG // 2):
                        psr = prep.tile([P, 1024], F32, tag="prep")
                        for h in (0, 1):
                            g = 2 * gp + h
                            nc.tensor.matmul(
                                psr[:, 512 * h : 512 * (h + 1)],
                                lhsT=eall[:, g, :],
                                rhs=xT[:, 4 * ib : 4 * ib + 4, :].rearrange(
                                    "c t p -> c (t p)"
                                ),
                                start=True,
                                stop=True,
                            )
                        nc.vector.tensor_mul(
                            outerT[:, 1024 * gp : 1024 * (gp + 1)],
                            psr,
                            s2dup[:, ib, :],
                        )
                    for t in range(4):
                        pso = pout.tile([P, D], F32, tag="pout")
                        for g in range(NG):
                            nc.tensor.matmul(
                                pso,
                                lhsT=outerT[
                                    :, 512 * g + 128 * t : 512 * g + 128 * (t + 1)
                                ],
                                rhs=w2t[:, g, :],
                                start=(g == 0),
                                stop=(g == NG - 1),
                            )
                        # epilogue: residual + LayerNorm
                        it = 4 * ib + t
                        r = eplg.tile([P, D], F32, tag="r")
                        nc.vector.tensor_add(r, pso, xf[:, it, :])
                        stats = eplg.tile([P, 6], F32, tag="stats")
                        nc.vector.bn_stats(out=stats, in_=r)
                        mv = eplg.tile([P, 2], F32, tag="mv")
                        nc.vector.bn_aggr(out=mv, in_=stats)
                        rstd = eplg.tile([P, 1], F32, tag="rstd")
                        nc.scalar.activation(
                            out=rstd,
                            in_=mv[:, 1:2],
                            func=mybir.ActivationFunctionType.Sqrt,
                            bias=eps_t,
                            scale=1.0,
                        )
                        nc.vector.reciprocal(out=rstd, in_=rstd)
                        y = eplg.tile([P, D], F32, tag="y")
                        nc.vector.tensor_scalar(
                            out=y,
                            in0=r,
                            scalar1=mv[:, 0:1],
                            scalar2=rstd,
                            op0=mybir.AluOpType.subtract,
                            op1=mybir.AluOpType.mult,
                        )
                        nc.vector.tensor_mul(y, y, gam)
                        nc.vector.tensor_add(y, y, bet)
                        nc.sync.dma_start(
                            out=y_d[128 * it : 128 * (it + 1), :], in_=y
                        )

            if reps == 1:
                body()
            else:
                with tc.For_i(0, reps, 1):
                    body()

    _split_multiwait(nc)
    return nc


def _make_in_maps(x, w, gamma, beta):
    tts, w2t, eall = _host_constants(w)
    in_maps = []
    for b in range(B):
        xb = np.ascontiguousarray(x[b])
        in_maps.append(
            {
                "xb": xb,
                "x2b": np.concatenate([xb, xb], axis=1).astype(BF16_NP),
                "tts": tts,
                "w2t": w2t,
                "eall": eall,
                "gamma": gamma,
                "beta": beta,
            }
        )
    return in_maps


_CACHED = {}


def kernel(**inputs: np.ndarray) -> np.ndarray:
    x = np.asarray(inputs["x"], np.float32)
    w = np.asarray(inputs["concept_map"], np.float32)
    gamma = np.asarray(inputs["gamma"], np.float32)
    beta = np.asarray(inputs["beta"], np.float32)
    assert x.shape == (B, S, D)

    if "nc" not in _CACHED:
        _CACHED["nc"] = _build_nc()
    nc = _CACHED["nc"]
    in_maps = _make_in_maps(x, w, gamma, beta)
    res = run_bass_kernel_spmd(nc, in_maps, core_ids=list(range(B)))
    return np.stack([res.results[b]["y"] for b in range(B)], axis=0)


if __name__ == "__main__":
    rng = np.random.default_rng(0)
    ins = {
        "x": rng.standard_normal((B, S, D), dtype=np.float32),
        "concept_map": (rng.standard_normal((D, D, D)) * 0.02).astype(np.float32),
        "gamma": np.ones(D, np.float32),
        "beta": np.zeros(D, np.float32),
    }
    y = kernel(**ins)
    print("ran", y.shape, y.dtype)


# revision 12
# speedup vs baseline: 6479.2066x; 1.1483x over previous
"""Trainium2 Bass kernel for nn_ConceptLayer (B=8, S=4096, D=64).

out[b,i,k] = LN( x[b,i,:] + sum_{a,c} x[b,i,a] * s_pre[b,i,c] * W[k,a,c] )
s_pre[b,i,c] = sum_{j<i} x[b,j,c] / (i-j)^2

Sharding: data-parallel over batch — one batch element per NeuronCore (8 cores).

Per-core algorithm (v3):
  One PSUM "megatile" (128, 4096) f32 spans all 8 banks; regions are carved
  manually (phases are sequential per region, Tile tracks subtile deps).

  Phase A (PE): s2[c(+dup), 512-block ib] = sum_J x2[J].T @ TTS-slice into
    megatile[:, 512*ib .. ] (Toeplitz strip, causal diag); DVE copies -> s2b
    (bf16, rows [c; c]).
  Phase B, per (a,c)-chunk g (128 rows, a-major: rows p -> a=2g+p//64, c=p%64):
    xrep_g[p, i] = x[i, 2g+p//64]: two broadcast DMAs from DRAM xT rows
      (partition-stride-0 source, full 4096-wide granule; sync+scalar rings)
    outerT_g = xrep_g * s2b           (DVE, bf16 2x mode)
    outT[k, u-slice] += W2T_g.T @ outerT_g[:, u-slice]   (PE, M=64 -> the
      whole (64, 4096) output accumulates in megatile[0:64, :], one open
      accumulation group per (partition-range x bank) region)
  Phase C: copy outT -> SBUF f32 (ACT); PE-transpose 128x64 tiles back to
    (i, k) into rotating megatile regions; epilogue per i-tile: residual add,
    LayerNorm over k (bn_stats/bn_aggr + sqrt + recip), gamma/beta (GPSIMD),
    DMA out.
"""

import sys

sys.path.insert(0, "/opt/trn_rl_repo")

import numpy as np
import ml_dtypes

import concourse.bass as bass
import concourse.mybir as mybir
from concourse.tile import TileContext
from concourse.bass_utils import run_bass_kernel_spmd
from concourse.masks import make_identity

B, S, D = 8, 4096, 64
LN_EPS = 1e-3
P = 128
NT = S // P            # 32 i-tiles
NB = S // 512          # 8 512-blocks
NG = (D * D) // P      # 32 (a,c) chunks
NSTRIP = NB * 4 + 3    # 35 offset blocks in the Toeplitz strip

F32 = mybir.dt.float32
BF16 = mybir.dt.bfloat16
BF16_NP = ml_dtypes.bfloat16


# ---------------------------------------------------------------------------
# Workaround for walrus "Too many sync wait commands": this walrus build only
# accepts a single embedded sem wait per instruction. After Tile scheduling,
# split any instruction with N>1 waits into N-1 single-wait NOPs (same engine,
# placed just before it — identical blocking semantics).
def _split_multiwait(nc: bass.Bass, keep: int = 1):
    n = 0
    for fn in nc.m.functions:
        for bb in fn.blocks:
            insts = list(bb.instructions)
            out = []
            changed = False
            for inst in insts:
                si = inst.sync_info
                if si is not None and len(si.on_wait) > keep:
                    waits = list(si.on_wait)
                    for w in waits[:-keep]:
                        nop = mybir.InstNoOp(
                            name=f"WSPLIT-{n}", engine=inst.engine, ins=[], outs=[]
                        )
                        n += 1
                        nop.sync_info = mybir.SyncInfo(on_wait=[w], on_update=[])
                        out.append(nop)
                    inst.sync_info = mybir.SyncInfo(
                        on_wait=waits[-keep:], on_update=list(si.on_update)
                    )
                    changed = True
                out.append(inst)
            if changed:
                bb.instructions = out
    return n
# ---------------------------------------------------------------------------


def _host_constants(concept_map: np.ndarray):
    """Precompute host-side constant tensors (replicated across cores)."""
    # Toeplitz strip: TTS[q, 128*s + n] = f(128*(s-3) + n - q), f(v)=1/v^2 (v>0)
    q = np.arange(P)
    col = np.arange(NSTRIP * P)
    sblk, n_ = col // P, col % P
    v = 128 * (sblk[None, :] - 3) + n_[None, :] - q[:, None]
    tts = np.where(v > 0, 1.0 / np.maximum(v, 1).astype(np.float64) ** 2, 0.0)
    tts = tts.astype(np.float32)

    # W2T[a*64+c, k] = W[k, a, c]
    w2t = np.ascontiguousarray(
        concept_map.transpose(1, 2, 0).reshape(D * D, D)
    ).astype(np.float32)
    return tts.astype(BF16_NP), w2t.astype(BF16_NP)


def _build_nc(reps: int = 1, split: bool = True) -> bass.Bass:
    nc = bass.Bass("TRN2", target_bir_lowering=False, debug=False, num_devices=B)

    xb = nc.dram_tensor("xb", [S, D], F32, kind="ExternalInput")
    x2b = nc.dram_tensor("x2b", [S, 2 * D], BF16, kind="ExternalInput")
    xtb = nc.dram_tensor("xtb", [D, S], BF16, kind="ExternalInput")
    tts_d = nc.dram_tensor("tts", [P, NSTRIP * P], BF16, kind="ExternalInput")
    w2t_d = nc.dram_tensor("w2t", [D * D, D], BF16, kind="ExternalInput")
    gamma_d = nc.dram_tensor("gamma", [D], F32, kind="ExternalInput")
    beta_d = nc.dram_tensor("beta", [D], F32, kind="ExternalInput")
    y_d = nc.dram_tensor("y", [S, D], F32, kind="ExternalOutput")

    dma_engs = [nc.sync, nc.scalar]

    with TileContext(nc) as tc:
        with (
            tc.tile_pool(name="singles", bufs=1) as singles,
            tc.tile_pool(name="xrep", bufs=3) as xrep_pool,
            tc.tile_pool(name="outp", bufs=3) as out_pool,
            tc.tile_pool(name="eplg", bufs=4) as eplg,
            tc.tile_pool(name="psum", bufs=1, space="PSUM") as psum,
        ):

            def body():
                # ---- resident SBUF tiles ---------------------------------
                xf = singles.tile([P, NT, D], F32, tag="xf")
                nc.sync.dma_start(out=xf, in_=xb.rearrange("(j p) c -> p j c", p=P))
                x2t = singles.tile([P, NT, 2 * D], BF16, tag="x2t")
                nc.sync.dma_start(
                    out=x2t, in_=x2b.rearrange("(j p) c -> p j c", p=P)
                )
                tts = singles.tile([P, NSTRIP * P], BF16, tag="tts")
                nc.sync.dma_start(out=tts, in_=tts_d[:])
                w2t = singles.tile([P, NG, D], BF16, tag="w2t")
                nc.sync.dma_start(
                    out=w2t, in_=w2t_d.rearrange("(g p) k -> p g k", p=P)
                )
                gam = singles.tile([P, D], F32, tag="gam")
                nc.sync.dma_start(
                    out=gam,
                    in_=bass.AP(
                        tensor=gamma_d.ap().tensor,
                        offset=gamma_d.ap().offset,
                        ap=[[0, P], [1, D]],
                    ),
                )
                bet = singles.tile([P, D], F32, tag="bet")
                nc.sync.dma_start(
                    out=bet,
                    in_=bass.AP(
                        tensor=beta_d.ap().tensor,
                        offset=beta_d.ap().offset,
                        ap=[[0, P], [1, D]],
                    ),
                )
                eps_t = singles.tile([P, 1], F32, tag="eps")
                nc.vector.memset(eps_t, LN_EPS)
                ident = singles.tile([P, P], F32, tag="ident")
                make_identity(nc, ident)

                s2b = singles.tile([P, S], BF16, tag="s2b")
                otb = singles.tile([D, S], F32, tag="otb")

                mega = psum.tile([P, S], F32, tag="mega")

                # ---- Phase A: s_pre (PE) into megatile -------------------
                for ib in range(NB):
                    asl = slice(512 * ib, 512 * (ib + 1))
                    for J in range(4 * ib + 4):
                        s0 = 4 * ib - J + 3
                        nc.tensor.matmul(
                            mega[:, asl],
                            lhsT=x2t[:, J, :],
                            rhs=tts[:, 128 * s0 : 128 * s0 + 512],
                            start=(J == 0),
                            stop=(J == 4 * ib + 3),
                        )
                    nc.vector.tensor_copy(out=s2b[:, asl], in_=mega[:, asl])

                # ---- Phase B: bcast, product, bilinear into outT gang ----
                for g in range(NG):
                    xr = xrep_pool.tile([P, S], BF16, tag="xrep")
                    for h in (0, 1):
                        row = 2 * g + h
                        src = xtb[row : row + 1, :]
                        src_b = bass.AP(
                            tensor=src.tensor,
                            offset=src.offset,
                            ap=[[0, D], [1, S]],
                        )
                        dma_engs[(2 * g + h) % 2].dma_start(
                            out=xr[D * h : D * (h + 1), :], in_=src_b
                        )
                    ot = out_pool.tile([P, S], BF16, tag="outerT")
                    nc.vector.tensor_mul(ot, xr, s2b)
                    for u in range(NB):
                        nc.tensor.matmul(
                            mega[0:D, 512 * u : 512 * (u + 1)],
                            lhsT=w2t[:, g, :],
                            rhs=ot[:, 512 * u : 512 * (u + 1)],
                            start=(g == 0),
                            stop=(g == NG - 1),
                        )

                # ---- Phase C: copy outT, transpose, LN epilogue ----------
                for u in range(NB):
                    nc.scalar.copy(
                        out=otb[:, 512 * u : 512 * (u + 1)],
                        in_=mega[0:D, 512 * u : 512 * (u + 1)],
                    )
                for t in range(NT):
                    bk = t % NB
                    tsl = slice(512 * bk, 512 * bk + D)
                    nc.tensor.transpose(
                        mega[:, tsl],
                        in_=otb[:, 128 * t : 128 * (t + 1)],
                        identity=ident[0:D, 0:D],
                    )
                    r = eplg.tile([P, D], F32, tag="r")
                    nc.vector.tensor_add(r, mega[:, tsl], xf[:, t, :])
                    stats = eplg.tile([P, 6], F32, tag="stats")
                    nc.vector.bn_stats(out=stats, in_=r)
                    mv = eplg.tile([P, 2], F32, tag="mv")
                    nc.vector.bn_aggr(out=mv, in_=stats)
                    rstd = eplg.tile([P, 1], F32, tag="rstd")
                    nc.scalar.activation(
                        out=rstd,
                        in_=mv[:, 1:2],
                        func=mybir.ActivationFunctionType.Sqrt,
                        bias=eps_t,
                        scale=1.0,
                    )
                    nc.vector.reciprocal(out=rstd, in_=rstd)
                    y = eplg.tile([P, D], F32, tag="y")
                    nc.vector.tensor_scalar(
                        out=y,
                        in0=r,
                        scalar1=mv[:, 0:1],
                        scalar2=rstd,
                        op0=mybir.AluOpType.subtract,
                        op1=mybir.AluOpType.mult,
                    )
                    nc.gpsimd.tensor_mul(y, y, gam)
                    nc.gpsimd.tensor_add(y, y, bet)
                    nc.sync.dma_start(out=y_d[128 * t : 128 * (t + 1), :], in_=y)

            if reps == 1:
                body()
            else:
                with tc.For_i(0, reps, 1):
                    body()

    if split:
        _split_multiwait(nc)
    return nc


def _make_in_maps(x, w, gamma, beta):
    tts, w2t = _host_constants(w)
    in_maps = []
    for b in range(B):
        xb = np.ascontiguousarray(x[b])
        in_maps.append(
            {
                "xb": xb,
                "x2b": np.concatenate([xb, xb], axis=1).astype(BF16_NP),
                "xtb": np.ascontiguousarray(xb.T).astype(BF16_NP),
                "tts": tts,
                "w2t": w2t,
                "gamma": gamma,
                "beta": beta,
            }
        )
    return in_maps


_CACHED = {}


def kernel(**inputs: np.ndarray) -> np.ndarray:
    x = np.asarray(inputs["x"], np.float32)
    w = np.asarray(inputs["concept_map"], np.float32)
    gamma = np.asarray(inputs["gamma"], np.float32)
    beta = np.asarray(inputs["beta"], np.float32)
    assert x.shape == (B, S, D)

    if "nc" not in _CACHED:
        _CACHED["nc"] = _build_nc()
    nc = _CACHED["nc"]
    in_maps = _make_in_maps(x, w, gamma, beta)
    res = run_bass_kernel_spmd(nc, in_maps, core_ids=list(range(B)))
    return np.stack([res.results[b]["y"] for b in range(B)], axis=0)


if __name__ == "__main__":
    rng = np.random.default_rng(0)
    ins = {
        "x": rng.standard_normal((B, S, D), dtype=np.float32),
        "concept_map": (rng.standard_normal((D, D, D)) * 0.02).astype(np.float32),
        "gamma": np.ones(D, np.float32),
        "beta": np.zeros(D, np.float32),
    }
    y = kernel(**ins)
    print("ran", y.shape, y.dtype)


# revision 15
# speedup vs baseline: 6721.1264x; 1.0373x over previous
"""Trainium2 Bass kernel for nn_ConceptLayer (B=8, S=4096, D=64).

out[b,i,k] = LN( x[b,i,:] + sum_{a,c} x[b,i,a] * s_pre[b,i,c] * W[k,a,c] )
s_pre[b,i,c] = sum_{j<i} x[b,j,c] / (i-j)^2

Sharding: data-parallel over batch — one batch element per NeuronCore (8 cores).

Per-core algorithm (v3):
  One PSUM "megatile" (128, 4096) f32 spans all 8 banks; regions are carved
  manually (phases are sequential per region, Tile tracks subtile deps).

  Phase A (PE): s2[c(+dup), 512-block ib] = sum_J x2[J].T @ TTS-slice into
    megatile[:, 512*ib .. ] (Toeplitz strip, causal diag); DVE copies -> s2b
    (bf16, rows [c; c]).
  Phase B, per (a,c)-chunk g (128 rows, a-major: rows p -> a=2g+p//64, c=p%64):
    xrep_g[p, i] = x[i, 2g+p//64]: two broadcast DMAs from DRAM xT rows
      (partition-stride-0 source, full 4096-wide granule; sync+scalar rings)
    outerT_g = xrep_g * s2b           (DVE, bf16 2x mode)
    outT[k, u-slice] += W2T_g.T @ outerT_g[:, u-slice]   (PE, M=64 -> the
      whole (64, 4096) output accumulates in megatile[0:64, :], one open
      accumulation group per (partition-range x bank) region)
  Phase C: copy outT -> SBUF f32 (ACT); PE-transpose 128x64 tiles back to
    (i, k) into rotating megatile regions; epilogue per i-tile: residual add,
    LayerNorm over k (bn_stats/bn_aggr + sqrt + recip), gamma/beta (GPSIMD),
    DMA out.
"""

import sys

sys.path.insert(0, "/opt/trn_rl_repo")

import numpy as np
import ml_dtypes

import concourse.bass as bass
import concourse.mybir as mybir
from concourse.tile import TileContext
from concourse.bass_utils import run_bass_kernel_spmd
from concourse.masks import make_identity

B, S, D = 8, 4096, 64
LN_EPS = 1e-3
P = 128
NT = S // P            # 32 i-tiles
NB = S // 512          # 8 512-blocks
NG = (D * D) // P      # 32 (a,c) chunks
NSTRIP = NB * 4 + 3    # 35 offset blocks in the Toeplitz strip

F32 = mybir.dt.float32
BF16 = mybir.dt.bfloat16
BF16_NP = ml_dtypes.bfloat16


# ---------------------------------------------------------------------------
# Workaround for walrus "Too many sync wait commands": this walrus build only
# accepts a single embedded sem wait per instruction. After Tile scheduling,
# split any instruction with N>1 waits into N-1 single-wait NOPs (same engine,
# placed just before it — identical blocking semantics).
def _split_multiwait(nc: bass.Bass, keep: int = 1):
    n = 0
    for fn in nc.m.functions:
        for bb in fn.blocks:
            insts = list(bb.instructions)
            out = []
            changed = False
            for inst in insts:
                si = inst.sync_info
                if si is not None and len(si.on_wait) > keep:
                    waits = list(si.on_wait)
                    for w in waits[:-keep]:
                        nop = mybir.InstNoOp(
                            name=f"WSPLIT-{n}", engine=inst.engine, ins=[], outs=[]
                        )
                        n += 1
                        nop.sync_info = mybir.SyncInfo(on_wait=[w], on_update=[])
                        out.append(nop)
                    inst.sync_info = mybir.SyncInfo(
                        on_wait=waits[-keep:], on_update=list(si.on_update)
                    )
                    changed = True
                out.append(inst)
            if changed:
                bb.instructions = out
    return n
# ---------------------------------------------------------------------------


def _host_constants(concept_map: np.ndarray):
    """Precompute host-side constant tensors (replicated across cores)."""
    # Toeplitz strip: TTS[q, 128*s + n] = f(128*(s-3) + n - q), f(v)=1/v^2 (v>0)
    q = np.arange(P)
    col = np.arange(NSTRIP * P)
    sblk, n_ = col // P, col % P
    v = 128 * (sblk[None, :] - 3) + n_[None, :] - q[:, None]
    tts = np.where(v > 0, 1.0 / np.maximum(v, 1).astype(np.float64) ** 2, 0.0)
    tts = tts.astype(np.float32)

    # W2T[a*64+c, k] = W[k, a, c]
    w2t = np.ascontiguousarray(
        concept_map.transpose(1, 2, 0).reshape(D * D, D)
    ).astype(np.float32)
    return tts.astype(BF16_NP), w2t.astype(BF16_NP)


def _build_nc(reps: int = 1, split: bool = True) -> bass.Bass:
    nc = bass.Bass("TRN2", target_bir_lowering=False, debug=False, num_devices=B)

    xb = nc.dram_tensor("xb", [S, D], F32, kind="ExternalInput")
    x2b = nc.dram_tensor("x2b", [S, 2 * D], BF16, kind="ExternalInput")
    xtb = nc.dram_tensor("xtb", [D, S], BF16, kind="ExternalInput")
    tts_d = nc.dram_tensor("tts", [P, NSTRIP * P], BF16, kind="ExternalInput")
    w2t_d = nc.dram_tensor("w2t", [D * D, D], BF16, kind="ExternalInput")
    gamma_d = nc.dram_tensor("gamma", [D], F32, kind="ExternalInput")
    beta_d = nc.dram_tensor("beta", [D], F32, kind="ExternalInput")
    y_d = nc.dram_tensor("y", [S, D], F32, kind="ExternalOutput")

    dma_engs = [nc.sync, nc.scalar]

    with TileContext(nc) as tc:
        with (
            tc.tile_pool(name="singles", bufs=1) as singles,
            tc.tile_pool(name="xrep", bufs=6) as xrep_pool,
            tc.tile_pool(name="outp", bufs=4) as out_pool,
            tc.tile_pool(name="eplg", bufs=4) as eplg,
            tc.tile_pool(name="psum", bufs=1, space="PSUM") as psum,
        ):

            def body():
                # ---- resident SBUF tiles ---------------------------------
                xf = singles.tile([P, NT, D], F32, tag="xf")
                nc.sync.dma_start(out=xf, in_=xb.rearrange("(j p) c -> p j c", p=P))
                x2t = singles.tile([P, NT, 2 * D], BF16, tag="x2t")
                nc.sync.dma_start(
                    out=x2t, in_=x2b.rearrange("(j p) c -> p j c", p=P)
                )
                tts = singles.tile([P, NSTRIP * P], BF16, tag="tts")
                nc.sync.dma_start(out=tts, in_=tts_d[:])
                w2t = singles.tile([P, NG, D], BF16, tag="w2t")
                nc.sync.dma_start(
                    out=w2t, in_=w2t_d.rearrange("(g p) k -> p g k", p=P)
                )
                gam = singles.tile([P, D], F32, tag="gam")
                nc.sync.dma_start(
                    out=gam,
                    in_=bass.AP(
                        tensor=gamma_d.ap().tensor,
                        offset=gamma_d.ap().offset,
                        ap=[[0, P], [1, D]],
                    ),
                )
                bet = singles.tile([P, D], F32, tag="bet")
                nc.sync.dma_start(
                    out=bet,
                    in_=bass.AP(
                        tensor=beta_d.ap().tensor,
                        offset=beta_d.ap().offset,
                        ap=[[0, P], [1, D]],
                    ),
                )
                eps_t = singles.tile([P, 1], F32, tag="eps")
                nc.vector.memset(eps_t, LN_EPS)
                ident = singles.tile([P, P], F32, tag="ident")
                make_identity(nc, ident)

                s2b = singles.tile([P, S], BF16, tag="s2b")
                otb = singles.tile([D, S], F32, tag="otb")

                mega = psum.tile([P, S], F32, tag="mega")

                # ---- Phase A: s_pre (PE) into megatile -------------------
                for ib in range(NB):
                    asl = slice(512 * ib, 512 * (ib + 1))
                    for J in range(4 * ib + 4):
                        s0 = 4 * ib - J + 3
                        nc.tensor.matmul(
                            mega[:, asl],
                            lhsT=x2t[:, J, :],
                            rhs=tts[:, 128 * s0 : 128 * s0 + 512],
                            start=(J == 0),
                            stop=(J == 4 * ib + 3),
                        )
                    nc.vector.tensor_copy(out=s2b[:, asl], in_=mega[:, asl])

                # ---- Phase B: bcast, product, bilinear into outT gang ----
                for g in range(NG):
                    xr = xrep_pool.tile([P, S], BF16, tag="xrep")
                    for h in (0, 1):
                        row = 2 * g + h
                        src = xtb[row : row + 1, :]
                        src_b = bass.AP(
                            tensor=src.tensor,
                            offset=src.offset,
                            ap=[[0, D], [1, S]],
                        )
                        dma_engs[(2 * g + h) % 2].dma_start(
                            out=xr[D * h : D * (h + 1), :], in_=src_b
                        )
                    ot = out_pool.tile([P, S], BF16, tag="outerT")
                    if g % 4 == 3:
                        nc.gpsimd.tensor_mul(ot, xr, s2b)
                    else:
                        nc.vector.tensor_mul(ot, xr, s2b)
                    for u in range(NB):
                        nc.tensor.matmul(
                            mega[0:D, 512 * u : 512 * (u + 1)],
                            lhsT=w2t[:, g, :],
                            rhs=ot[:, 512 * u : 512 * (u + 1)],
                            start=(g == 0),
                            stop=(g == NG - 1),
                        )

                # ---- Phase C: copy outT, transpose, LN epilogue ----------
                for u in range(NB):
                    nc.scalar.copy(
                        out=otb[:, 512 * u : 512 * (u + 1)],
                        in_=mega[0:D, 512 * u : 512 * (u + 1)],
                    )
                for t in range(NT):
                    bk = t % NB
                    tsl = slice(512 * bk, 512 * bk + D)
                    nc.tensor.transpose(
                        mega[:, tsl],
                        in_=otb[:, 128 * t : 128 * (t + 1)],
                        identity=ident[0:D, 0:D],
                    )
                    r = eplg.tile([P, D], F32, tag="r")
                    nc.vector.tensor_add(r, mega[:, tsl], xf[:, t, :])
                    stats = eplg.tile([P, 6], F32, tag="stats")
                    nc.vector.bn_stats(out=stats, in_=r)
                    mv = eplg.tile([P, 2], F32, tag="mv")
                    nc.vector.bn_aggr(out=mv, in_=stats)
                    rstd = eplg.tile([P, 1], F32, tag="rstd")
                    nc.scalar.activation(
                        out=rstd,
                        in_=mv[:, 1:2],
                        func=mybir.ActivationFunctionType.Sqrt,
                        bias=eps_t,
                        scale=1.0,
                    )
                    nc.vector.reciprocal(out=rstd, in_=rstd)
                    negmr = eplg.tile([P, 1], F32, tag="negmr")
                    nc.vector.tensor_scalar(
                        out=negmr,
                        in0=mv[:, 0:1],
                        scalar1=rstd,
                        scalar2=-1.0,
                        op0=mybir.AluOpType.mult,
                        op1=mybir.AluOpType.mult,
                    )
                    y = eplg.tile([P, D], F32, tag="y")
                    nc.scalar.activation(
                        out=y,
                        in_=r,
                        func=mybir.ActivationFunctionType.Identity,
                        bias=negmr,
                        scale=rstd,
                    )
                    nc.gpsimd.tensor_mul(y, y, gam)
                    nc.gpsimd.tensor_add(y, y, bet)
                    nc.sync.dma_start(out=y_d[128 * t : 128 * (t + 1), :], in_=y)

            if reps == 1:
                body()
            else:
                with tc.For_i(0, reps, 1):
                    body()

    if split:
        _split_multiwait(nc)
    return nc


def _make_in_maps(x, w, gamma, beta):
    tts, w2t = _host_constants(w)
    in_maps = []
    for b in range(B):
        xb = np.ascontiguousarray(x[b])
        in_maps.append(
            {
                "xb": xb,
                "x2b": np.concatenate([xb, xb], axis=1).astype(BF16_NP),
                "xtb": np.ascontiguousarray(xb.T).astype(BF16_NP),
                "tts": tts,
                "w2t": w2t,
                "gamma": gamma,
                "beta": beta,
            }
        )
    return in_maps


_CACHED = {}


def kernel(**inputs: np.ndarray) -> np.ndarray:
    x = np.asarray(inputs["x"], np.float32)
    w = np.asarray(inputs["concept_map"], np.float32)
    gamma = np.asarray(inputs["gamma"], np.float32)
    beta = np.asarray(inputs["beta"], np.float32)
    assert x.shape == (B, S, D)

    if "nc" not in _CACHED:
        _CACHED["nc"] = _build_nc()
    nc = _CACHED["nc"]
    in_maps = _make_in_maps(x, w, gamma, beta)
    res = run_bass_kernel_spmd(nc, in_maps, core_ids=list(range(B)))
    return np.stack([res.results[b]["y"] for b in range(B)], axis=0)


if __name__ == "__main__":
    rng = np.random.default_rng(0)
    ins = {
        "x": rng.standard_normal((B, S, D), dtype=np.float32),
        "concept_map": (rng.standard_normal((D, D, D)) * 0.02).astype(np.float32),
        "gamma": np.ones(D, np.float32),
        "beta": np.zeros(D, np.float32),
    }
    y = kernel(**ins)
    print("ran", y.shape, y.dtype)
